# revision 47
# baseline (speedup 1.0000x reference)
"""Binary linear layer (sign(x) @ sign(w)) on 8 trn2 NeuronCores.

Strategy
--------
Data-parallel: x is split into 8 row-blocks of 1024; the 4096x4096 weight is
replicated. Each core computes out_shard = sign(x_shard) @ sign(w).

All products are +/-1 and row sums are integers <= 4096, so the matmul is
exact in low precision with fp32 PSUM accumulation. The fast path ("fp8dr"):

- Host re-encodes both inputs to fp8e4 (sign-exact for every input value --
  see _encode_fp8) and pre-transposes each x shard to [d_in, n_per] so the
  PE contraction dim lands on SBUF partitions. 21 MB HBM in per core.
- Device binarizes x -> +/-1 (ACT engine, Sign) and w -> +/-0.5 (DVE,
  (v>=0)-0.5, in place), then runs fp8 DoubleRow matmuls (2 virtual PE rows
  per cell = 157 TFLOP/s): products +/-0.5, integer-scaled sums, all exact.
- The PSUM->SBUF eviction copy multiplies by 2 (exact power of two).
  Result is bit-identical to the fp32 reference.

n-chunk 0 runs kt-outer across all 8 PSUM banks so the PE paces behind the
streaming x DMA; later chunks run mt-outer with staggered evictions.
Measured ~248 us/kernel (fp8 roofline for the per-core GEMM is ~219 us).
"""

import numpy as np
import ml_dtypes

N_TOTAL, D_IN, D_OUT = 8192, 4096, 4096
N_CORES = 8
N_PER = N_TOTAL // N_CORES


def fp8_in(mode):
    return mode == "fp8dr"

# "bf16": bf16 matmul (exact).  "fp8dr": fp8e4 DoubleRow matmul (exact, 2
# virtual PE rows per cell -> half the matmul instructions).
MODE = "fp8dr"

_PROGRAM_CACHE = {}


def build_program(n_per=N_PER, d_in=D_IN, d_out=D_OUT, num_devices=N_CORES,
                  mode=None):
    """Build + compile the SPMD Bass program (same program on every core)."""
    from concourse import bacc, mybir, tile
    from concourse.bass import ds

    if mode is None:
        mode = MODE
    BF = mybir.dt.bfloat16
    F32 = mybir.dt.float32
    FP8 = mybir.dt.float8e4
    MMDT = FP8 if mode == "fp8dr" else BF  # matmul operand dtype
    P = 128
    NW = 512  # n-chunk width = one PSUM bank of fp32
    KT = d_in // P      # k-tiles
    MT = n_per // P     # m-tiles per core
    NCH = d_out // NW   # n-chunks
    ge = mybir.AluOpType.is_ge
    sub = mybir.AluOpType.subtract
    Copy = mybir.ActivationFunctionType.Copy
    Sign = mybir.ActivationFunctionType.Sign
    perf_mode = mybir.MatmulPerfMode.DoubleRow if mode == "fp8dr" else None
    # Engine for the x binarize in fp8 mode:
    #   "act"    -> ACT Sign, x = +/-1, copy scale 2
    #   "gpsimd" -> GpSimd (v>=0)-0.5, x = +/-0.5, copy scale 4
    # (w is always +/-0.5 on DVE; host patched +/-0 to +/-1 so either
    # binarizer reproduces sign(v) exactly.)
    XBIN = "act"  # "gpsimd" measured 3x slower end-to-end; keep ACT Sign
    if mode == "fp8dr":
        OUT_SCALE = 4.0 if XBIN == "gpsimd" else 2.0
    else:
        OUT_SCALE = 4.0

    nc = bacc.Bacc(
        "TRN2",
        target_bir_lowering=False,
        debug=False,
        enable_asserts=False,
        num_devices=num_devices,
    )
    # fp8 mode ships inputs as fp8e4 (host re-encode is sign-exact; see
    # shard_inputs) -> half the HBM traffic of bf16.
    INDT = FP8 if fp8_in(mode) else BF
    xt = nc.declare_dram_parameter("xt", [d_in, n_per], INDT, isOutput=False)
    w = nc.declare_dram_parameter("w", [d_in, d_out], INDT, isOutput=False)
    out = nc.declare_dram_parameter("out", [n_per, d_out], F32, isOutput=True)

    # HBM-side access patterns with the k-tile index folded into partitions.
    xt_r = xt.ap().rearrange("(kt p) m -> p kt m", p=P)        # [128, KT, n_per]
    w_r = w.ap().rearrange("(kt p) n -> p kt n", p=P)          # [128, KT, d_out]

    fp8 = mode == "fp8dr"
    if fp8:
        assert KT % 2 == 0

    with tile.TileContext(nc) as tc:
        with (
            tc.tile_pool(name="xpool", bufs=1) as xpool,
            tc.tile_pool(name="wpool", bufs=4 if fp8 else 2) as wpool,
            tc.tile_pool(name="opool", bufs=8) as opool,
            tc.tile_pool(name="psum", bufs=8, space="PSUM") as pspool,
        ):
            xb = xpool.tile([P, KT * n_per], MMDT, tag="xb")
            xb3 = xb[:, :].rearrange("p (kt m) -> p kt m", kt=KT)
            X_CH = min(16, KT)
            kt_per = KT // X_CH

            def x_dma(c, issue_engine=None):
                ktsl = ds(c * kt_per, kt_per)
                eng = issue_engine if issue_engine is not None else nc.sync
                eng.dma_start(out=xb3[:, ktsl, :], in_=xt_r[:, ktsl, :])

            def x_bin(c):
                fsl = ds(c * kt_per * n_per, kt_per * n_per)
                if fp8 and XBIN == "act":
                    # ACT engine; host patched exact zeros so Sign == (v>=0)
                    nc.scalar.activation(xb[:, fsl], xb[:, fsl], Sign)
                elif fp8 and XBIN == "gpsimd":
                    nc.gpsimd.tensor_scalar(
                        xb[:, fsl], xb[:, fsl], 0.0, 0.5, ge, sub
                    )
                else:
                    nc.vector.tensor_scalar(
                        xb[:, fsl], xb[:, fsl], 0.0, 0.5, ge, sub
                    )

            def load_x_chunk(c, issue_engine=None):
                x_dma(c, issue_engine)
                x_bin(c)

            HALF = max(1, KT // 2)
            N_HALVES = KT // HALF
            BIN_KT = min(4, HALF)  # k-tiles per DVE binarize op

            def load_w_chunk(nt, half, n_dmas=1):
                """DMA + binarize (in place) one k-half of w n-chunk nt."""
                nsl = ds(nt * NW, NW)
                wb = w_tiles[nt]
                wb3 = wb[:, :].rearrange("p (kt n) -> p kt n", kt=KT)
                n_dmas = min(n_dmas, HALF)
                per = HALF // n_dmas
                for d in range(n_dmas):
                    hsl = ds(half * HALF + d * per, per)
                    nc.sync.dma_start(
                        out=wb3[:, hsl, :], in_=w_r[:, hsl, nsl]
                    )
                for c in range(HALF // BIN_KT):
                    sl = ds((half * HALF + c * BIN_KT) * NW, BIN_KT * NW)
                    nc.vector.tensor_scalar(
                        wb[:, sl], wb[:, sl], 0.0, 0.5, ge, sub
                    )

            def alloc_w_tiles(nt):
                wb = wpool.tile([P, KT * NW], MMDT, tag="wb", name=f"wb{nt}")
                w_tiles[nt] = wb

            def mm(ps, mt, t, wb3, start, stop):
                if fp8:
                    nc.tensor.matmul(
                        ps[:, :],
                        lhsT=xb3[:, 2 * t : 2 * t + 2, ds(mt * P, P)],
                        rhs=wb3[:, 2 * t : 2 * t + 2, :],
                        start=start, stop=stop, perf_mode=perf_mode,
                    )
                else:
                    nc.tensor.matmul(
                        ps[:, :],
                        lhsT=xb[:, ds(t * n_per + mt * P, P)],
                        rhs=wb3[:, t, :],
                        start=start, stop=stop,
                    )

            def evict(ps, mt, nt):
                ot = opool.tile([P, NW], F32, tag="ot")
                nc.scalar.activation(ot[:, :], ps[:, :], Copy, 0.0, OUT_SCALE)
                nc.sync.dma_start(
                    out=out[ds(mt * P, P), ds(nt * NW, NW)], in_=ot[:, :]
                )

            w_tiles = {}
            NK = KT // 2 if fp8 else KT  # MM k-iterations per psum group

            ps0 = [
                pspool.tile([P, NW], F32, tag="ps", name=f"ps0_{i}")
                for i in range(MT)
            ]

            # HAM warmup via dummy matmuls measured neutral (chunk 0 is
            # DMA-paced, cold matmuls hide behind data arrival), and the PE
            # queue slots are more valuable as wait-free DMA-issue slots
            # below. Keep the path available but disabled.
            WARM_MMS = 0
            if WARM_MMS:
                warm = xpool.tile([P, P], MMDT, tag="warm", name="warm")
                nc.gpsimd.memset(warm[:, :], 1.0)
                for _ in range(WARM_MMS):
                    nc.tensor.matmul(
                        ps0[0][:, :P], lhsT=warm[:, :], rhs=warm[:, :],
                        start=True, stop=True,
                    )

            # Startup interleave: first half of w chunk 0, then x, then the
            # rest of w chunk 0 — so the PE can start at the first x k-tiles
            # and never waits on the second w half.
            # The sync sequencer issues DMAs serially at ~0.3-2us each, so a
            # 20-deep startup burst delays later transfers. Split the x-DMA
            # issues between sync and ACT: ACT's issue ops are emitted
            # BEFORE its Sign chain, so they run wait-free during the first
            # transfers. Binarize ops follow in arrival order.
            alloc_w_tiles(0)
            x_dma(0)
            load_w_chunk(0, 0, n_dmas=2 if HALF >= 8 else 1)
            if X_CH > 1:
                x_dma(1)
            mid = max(2, X_CH // 2)
            for c in range(2, mid):
                x_dma(c, issue_engine=nc.scalar)
            if N_HALVES > 1:
                load_w_chunk(0, 1)
            for c in range(mid, X_CH):
                x_dma(c)
            for c in range(X_CH):
                x_bin(c)

            # n-chunk 0: kt-outer across all MT psum banks, pacing the PE
            # behind the streaming x DMA instead of stalling on full x.
            wb3_0 = w_tiles[0][:, :].rearrange("p (kt n) -> p kt n", kt=KT)
            for t in range(NK):
                for mt in range(MT):
                    mm(ps0[mt], mt, t, wb3_0, start=(t == 0), stop=(t == NK - 1))
            for mt in range(MT):
                evict(ps0[mt], mt, 0)

            # n-chunks 1..: mt-outer (staggered psum eviction)
            for nt in range(1, NCH):
                alloc_w_tiles(nt)
                for h in range(N_HALVES):
                    load_w_chunk(nt, h)
                wb3 = w_tiles[nt][:, :].rearrange(
                    "p (kt n) -> p kt n", kt=KT
                )
                for mt in range(MT):
                    ps = pspool.tile([P, NW], F32, tag="ps")
                    for t in range(NK):
                        mm(ps, mt, t, wb3, start=(t == 0), stop=(t == NK - 1))
                    evict(ps, mt, nt)

    nc.compile()
    return nc


def _get_program():
    key = (N_PER, D_IN, D_OUT, MODE)
    if key not in _PROGRAM_CACHE:
        _PROGRAM_CACHE[key] = build_program()
    return _PROGRAM_CACHE[key]


def _encode_fp8(v):
    """Sign-exact fp8e4 re-encode of fp32 data for the device binarizer.

    ml_dtypes.float8_e4m3 matches TRN FP8_EXP4 (max 240, overflow saturates
    to +/-Inf, underflow to +/-0 -- sign always survives in the result).
    The only sign-ambiguous encodings are +/-0, which we patch to +/-1:
    +0 covers true zeros (reference maps them to +1) and underflowed
    positives; -0 covers underflowed negatives. After the patch the device
    binarize (v >= 0, or Sign) reproduces sign(original fp32) exactly for
    EVERY possible input value.
    """
    f8 = ml_dtypes.float8_e4m3
    v8 = np.clip(v, -240.0, 240.0).astype(f8)
    z = v8 == 0
    if z.any():
        v8 = np.where(z, np.where(np.signbit(v8), -1.0, 1.0).astype(f8), v8)
    return v8


def shard_inputs(x, weight):
    """Host-side sharding/layout: dtype re-encode + per-shard transpose."""
    if fp8_in(MODE):
        xe = _encode_fp8(x)
        we = _encode_fp8(weight)
    else:
        bf16 = ml_dtypes.bfloat16
        xe = x.astype(bf16)
        we = weight.astype(bf16)
    we = np.ascontiguousarray(we)
    shards = [
        np.ascontiguousarray(xe[i * N_PER : (i + 1) * N_PER].T)
        for i in range(N_CORES)
    ]
    return [{"xt": shards[i], "w": we} for i in range(N_CORES)]


def kernel(x, weight):
    from concourse.bass_utils import run_bass_kernel_spmd

    nc = _get_program()
    in_maps = shard_inputs(np.asarray(x), np.asarray(weight))
    res = run_bass_kernel_spmd(nc, in_maps, list(range(N_CORES)))
    return np.concatenate(
        [res.results[i]["out"] for i in range(N_CORES)], axis=0
    )


# revision 49
# speedup vs baseline: 1.0229x; 1.0229x over previous
"""Binary linear layer (sign(x) @ sign(w)) on 8 trn2 NeuronCores.

Strategy
--------
Data-parallel: x is split into 8 row-blocks of 1024; the 4096x4096 weight is
replicated. Each core computes out_shard = sign(x_shard) @ sign(w).

All products are +/-1 and row sums are integers <= 4096, so the matmul is
exact in low precision with fp32 PSUM accumulation. The fast path ("fp8dr"):

- Host re-encodes both inputs to fp8e4 (sign-exact for every input value --
  see _encode_fp8) and pre-transposes each x shard to [d_in, n_per] so the
  PE contraction dim lands on SBUF partitions. 21 MB HBM in per core.
- Device binarizes x -> +/-1 (ACT engine, Sign) and w -> +/-0.5 (DVE,
  (v>=0)-0.5, in place), then runs fp8 DoubleRow matmuls (2 virtual PE rows
  per cell = 157 TFLOP/s): products +/-0.5, integer-scaled sums, all exact.
- The PSUM->SBUF eviction copy multiplies by 2 (exact power of two).
  Result is bit-identical to the fp32 reference.

n-chunk 0 runs kt-outer across all 8 PSUM banks so the PE paces behind the
streaming x DMA; later chunks run mt-outer with staggered evictions.
Measured ~248 us/kernel (fp8 roofline for the per-core GEMM is ~219 us).
"""

import numpy as np
import ml_dtypes

N_TOTAL, D_IN, D_OUT = 8192, 4096, 4096
N_CORES = 8
N_PER = N_TOTAL // N_CORES


def fp8_in(mode):
    return mode == "fp8dr"

# "bf16": bf16 matmul (exact).  "fp8dr": fp8e4 DoubleRow matmul (exact, 2
# virtual PE rows per cell -> half the matmul instructions).
MODE = "fp8dr"

_PROGRAM_CACHE = {}


def build_program(n_per=N_PER, d_in=D_IN, d_out=D_OUT, num_devices=N_CORES,
                  mode=None):
    """Build + compile the SPMD Bass program (same program on every core)."""
    from concourse import bacc, mybir, tile
    from concourse.bass import ds

    if mode is None:
        mode = MODE
    BF = mybir.dt.bfloat16
    F32 = mybir.dt.float32
    FP8 = mybir.dt.float8e4
    MMDT = FP8 if mode == "fp8dr" else BF  # matmul operand dtype
    P = 128
    NW = 512  # n-chunk width = one PSUM bank of fp32
    KT = d_in // P      # k-tiles
    MT = n_per // P     # m-tiles per core
    NCH = d_out // NW   # n-chunks
    ge = mybir.AluOpType.is_ge
    sub = mybir.AluOpType.subtract
    Copy = mybir.ActivationFunctionType.Copy
    Sign = mybir.ActivationFunctionType.Sign
    perf_mode = mybir.MatmulPerfMode.DoubleRow if mode == "fp8dr" else None
    # Engine for the x binarize in fp8 mode:
    #   "act"    -> ACT Sign, x = +/-1, copy scale 2
    #   "gpsimd" -> GpSimd (v>=0)-0.5, x = +/-0.5, copy scale 4
    # (w is always +/-0.5 on DVE; host patched +/-0 to +/-1 so either
    # binarizer reproduces sign(v) exactly.)
    XBIN = "act"  # "gpsimd" measured 3x slower end-to-end; keep ACT Sign
    if mode == "fp8dr":
        OUT_SCALE = 4.0 if XBIN == "gpsimd" else 2.0
    else:
        OUT_SCALE = 4.0

    nc = bacc.Bacc(
        "TRN2",
        target_bir_lowering=False,
        debug=False,
        enable_asserts=False,
        num_devices=num_devices,
    )
    # fp8 mode ships inputs as fp8e4 (host re-encode is sign-exact; see
    # shard_inputs) -> half the HBM traffic of bf16.
    INDT = FP8 if fp8_in(mode) else BF
    xt = nc.declare_dram_parameter("xt", [d_in, n_per], INDT, isOutput=False)
    w = nc.declare_dram_parameter("w", [d_in, d_out], INDT, isOutput=False)
    out = nc.declare_dram_parameter("out", [n_per, d_out], F32, isOutput=True)

    # HBM-side access patterns with the k-tile index folded into partitions.
    xt_r = xt.ap().rearrange("(kt p) m -> p kt m", p=P)        # [128, KT, n_per]
    w_r = w.ap().rearrange("(kt p) n -> p kt n", p=P)          # [128, KT, d_out]

    fp8 = mode == "fp8dr"
    if fp8:
        assert KT % 2 == 0

    with tile.TileContext(nc) as tc:
        with (
            tc.tile_pool(name="xpool", bufs=1) as xpool,
            tc.tile_pool(name="wpool", bufs=4 if fp8 else 2) as wpool,
            tc.tile_pool(name="opool", bufs=8) as opool,
            tc.tile_pool(name="psum", bufs=8, space="PSUM") as pspool,
        ):
            xb = xpool.tile([P, KT * n_per], MMDT, tag="xb")
            xb3 = xb[:, :].rearrange("p (kt m) -> p kt m", kt=KT)
            X_CH = min(16, KT)
            kt_per = KT // X_CH

            def x_dma(c, issue_engine=None):
                ktsl = ds(c * kt_per, kt_per)
                eng = issue_engine if issue_engine is not None else nc.sync
                eng.dma_start(out=xb3[:, ktsl, :], in_=xt_r[:, ktsl, :])

            def x_bin(c):
                fsl = ds(c * kt_per * n_per, kt_per * n_per)
                if fp8 and XBIN == "act":
                    # ACT engine; host patched exact zeros so Sign == (v>=0)
                    nc.scalar.activation(xb[:, fsl], xb[:, fsl], Sign)
                elif fp8 and XBIN == "gpsimd":
                    nc.gpsimd.tensor_scalar(
                        xb[:, fsl], xb[:, fsl], 0.0, 0.5, ge, sub
                    )
                else:
                    nc.vector.tensor_scalar(
                        xb[:, fsl], xb[:, fsl], 0.0, 0.5, ge, sub
                    )

            def load_x_chunk(c, issue_engine=None):
                x_dma(c, issue_engine)
                x_bin(c)

            HALF = max(1, KT // 2)
            N_HALVES = KT // HALF
            BIN_KT = min(4, HALF)  # k-tiles per DVE binarize op

            def load_w_chunk(nt, half, n_dmas=1):
                """DMA + binarize (in place) one k-half of w n-chunk nt."""
                nsl = ds(nt * NW, NW)
                wb = w_tiles[nt]
                wb3 = wb[:, :].rearrange("p (kt n) -> p kt n", kt=KT)
                n_dmas = min(n_dmas, HALF)
                per = HALF // n_dmas
                for d in range(n_dmas):
                    hsl = ds(half * HALF + d * per, per)
                    nc.sync.dma_start(
                        out=wb3[:, hsl, :], in_=w_r[:, hsl, nsl]
                    )
                for c in range(HALF // BIN_KT):
                    sl = ds((half * HALF + c * BIN_KT) * NW, BIN_KT * NW)
                    nc.vector.tensor_scalar(
                        wb[:, sl], wb[:, sl], 0.0, 0.5, ge, sub
                    )

            def alloc_w_tiles(nt):
                wb = wpool.tile([P, KT * NW], MMDT, tag="wb", name=f"wb{nt}")
                w_tiles[nt] = wb

            def mm(ps, mt, t, wb3, start, stop):
                if fp8:
                    nc.tensor.matmul(
                        ps[:, :],
                        lhsT=xb3[:, 2 * t : 2 * t + 2, ds(mt * P, P)],
                        rhs=wb3[:, 2 * t : 2 * t + 2, :],
                        start=start, stop=stop, perf_mode=perf_mode,
                    )
                else:
                    nc.tensor.matmul(
                        ps[:, :],
                        lhsT=xb[:, ds(t * n_per + mt * P, P)],
                        rhs=wb3[:, t, :],
                        start=start, stop=stop,
                    )

            def evict(ps, mt, nt):
                ot = opool.tile([P, NW], F32, tag="ot")
                nc.scalar.activation(ot[:, :], ps[:, :], Copy, 0.0, OUT_SCALE)
                nc.sync.dma_start(
                    out=out[ds(mt * P, P), ds(nt * NW, NW)], in_=ot[:, :]
                )

            w_tiles = {}
            NK = KT // 2 if fp8 else KT  # MM k-iterations per psum group

            ps0 = [
                pspool.tile([P, NW], F32, tag="ps", name=f"ps0_{i}")
                for i in range(MT)
            ]

            # HAM warmup: the PE is idle for ~8us while the first DMAs land,
            # and the activity monitor keeps a cold PE at half clock for the
            # first ~3.4us of work. Burn that idle time with dummy matmuls on
            # a memset tile (into ps0[0], which the real k-group overwrites
            # with start=True) so real matmuls start at full clock.
            WARM_MMS = 80 if KT >= 16 else 8
            if WARM_MMS:
                warm = xpool.tile([P, P], MMDT, tag="warm", name="warm")
                nc.gpsimd.memset(warm[:, :], 1.0)
                for _ in range(WARM_MMS):
                    nc.tensor.matmul(
                        ps0[0][:, :P], lhsT=warm[:, :], rhs=warm[:, :],
                        start=True, stop=True,
                    )

            # Startup interleave: first half of w chunk 0, then x, then the
            # rest of w chunk 0 — so the PE can start at the first x k-tiles
            # and never waits on the second w half.
            # Startup interleave: first half of w chunk 0, then x, then the
            # rest of w chunk 0 — so the PE can start at the first x k-tiles
            # and never waits on the second w half. (Offloading x-DMA issues
            # to ACT's queue was tried and measured ~4.5us WORSE: it delays
            # the Sign chain more than it relieves the sync sequencer.)
            alloc_w_tiles(0)
            load_x_chunk(0)
            load_w_chunk(0, 0, n_dmas=2 if HALF >= 8 else 1)
            for c in range(1, X_CH // 2):
                load_x_chunk(c)
            if N_HALVES > 1:
                load_w_chunk(0, 1)
            for c in range(X_CH // 2, X_CH):
                load_x_chunk(c)

            # n-chunk 0: kt-outer across all MT psum banks, pacing the PE
            # behind the streaming x DMA instead of stalling on full x.
            wb3_0 = w_tiles[0][:, :].rearrange("p (kt n) -> p kt n", kt=KT)
            for t in range(NK):
                for mt in range(MT):
                    mm(ps0[mt], mt, t, wb3_0, start=(t == 0), stop=(t == NK - 1))
            for mt in range(MT):
                evict(ps0[mt], mt, 0)

            # n-chunks 1..: mt-outer (staggered psum eviction)
            for nt in range(1, NCH):
                alloc_w_tiles(nt)
                for h in range(N_HALVES):
                    load_w_chunk(nt, h)
                wb3 = w_tiles[nt][:, :].rearrange(
                    "p (kt n) -> p kt n", kt=KT
                )
                for mt in range(MT):
                    ps = pspool.tile([P, NW], F32, tag="ps")
                    for t in range(NK):
                        mm(ps, mt, t, wb3, start=(t == 0), stop=(t == NK - 1))
                    evict(ps, mt, nt)

    nc.compile()
    return nc


def _get_program():
    key = (N_PER, D_IN, D_OUT, MODE)
    if key not in _PROGRAM_CACHE:
        _PROGRAM_CACHE[key] = build_program()
    return _PROGRAM_CACHE[key]


def _encode_fp8(v):
    """Sign-exact fp8e4 re-encode of fp32 data for the device binarizer.

    ml_dtypes.float8_e4m3 matches TRN FP8_EXP4 (max 240, overflow saturates
    to +/-Inf, underflow to +/-0 -- sign always survives in the result).
    The only sign-ambiguous encodings are +/-0, which we patch to +/-1:
    +0 covers true zeros (reference maps them to +1) and underflowed
    positives; -0 covers underflowed negatives. After the patch the device
    binarize (v >= 0, or Sign) reproduces sign(original fp32) exactly for
    EVERY possible input value.
    """
    f8 = ml_dtypes.float8_e4m3
    v8 = np.clip(v, -240.0, 240.0).astype(f8)
    z = v8 == 0
    if z.any():
        v8 = np.where(z, np.where(np.signbit(v8), -1.0, 1.0).astype(f8), v8)
    return v8


def shard_inputs(x, weight):
    """Host-side sharding/layout: dtype re-encode + per-shard transpose."""
    if fp8_in(MODE):
        xe = _encode_fp8(x)
        we = _encode_fp8(weight)
    else:
        bf16 = ml_dtypes.bfloat16
        xe = x.astype(bf16)
        we = weight.astype(bf16)
    we = np.ascontiguousarray(we)
    shards = [
        np.ascontiguousarray(xe[i * N_PER : (i + 1) * N_PER].T)
        for i in range(N_CORES)
    ]
    return [{"xt": shards[i], "w": we} for i in range(N_CORES)]


def kernel(x, weight):
    from concourse.bass_utils import run_bass_kernel_spmd

    nc = _get_program()
    in_maps = shard_inputs(np.asarray(x), np.asarray(weight))
    res = run_bass_kernel_spmd(nc, in_maps, list(range(N_CORES)))
    return np.concatenate(
        [res.results[i]["out"] for i in range(N_CORES)], axis=0
    )
